# revision 12
# baseline (speedup 1.0000x reference)
"""Trainium2 Bass kernel for DigitConvolutionalModel forward pass.

Model: x[B,784] -> 3x3 valid conv (single channel) -> flatten[676]
       -> relu(.@W1+b1) -> relu(.@W2+b2) -> .@W3+b3 -> [B,10]

Strategy (v4):
  - Pure data parallel: batch 32768 sharded 8 ways (4096 rows/core);
    weights replicated.
  - conv folds into fc1 (host-side 9-tap sparse weight fold, ~0.02% of
    model FLOPs): fc1 contracts K=784 of pixel-major x against
    W1' = C @ W1. All batch compute runs on device in bf16 (fp32 PSUM).
  - Host supplies x pixel-major bf16 ([784, 4096] per core) and reads the
    output back pixel-major ([10, 4096] per core) — zero-FLOP layout
    changes that remove every on-device transpose.
  - fc1's K=16 leftover chunk (784 = 6*128 + 16) is packed: the three
    h-group tail matmuls run concurrently in disjoint 32-row PE groups
    (tile_position), with x[768:784] and W1'[768:784] replicated at
    partition offsets 0/32/64.
  - fc3 keeps hidden-major [10, 512] output (stationary = W3 chunks of
    only 10 columns -> LDWEIGHTS ~free); bias fused in the ScalarE
    eviction; the tile DMAs straight out to the [10, 4096] buffer.
  - Input + weight DMAs split across both HW-DGE rings (SP + Activation)
    so the prologue is not serialized behind one ~200 GB/s queue.
"""

import sys

for _p in (
    "/opt/trn_rl_repo",
    "/root/.axon_site",
    "/root/.axon_site/_ro/trn_rl_repo",
    "/root/.axon_site/_ro/pypackages",
):
    if _p not in sys.path:
        sys.path.append(_p)

from contextlib import ExitStack

import numpy as np
import ml_dtypes

import concourse.bass as bass
import concourse.tile as tile
from concourse import mybir
from concourse.bass_utils import run_bass_kernel_spmd

F32 = mybir.dt.float32
BF16 = mybir.dt.bfloat16
AFT = mybir.ActivationFunctionType

B_FULL = 32768
N_CORES = 8
B_CORE = B_FULL // N_CORES  # 4096
IMG = 28
OHW = 26
FLAT = OHW * OHW  # 676
NPIX = IMG * IMG  # 784
HID = 300
NCLS = 10

BT = 512  # batch tile (matmul moving free dim)
NBT = B_CORE // BT  # 8

NFULL = 6  # full 128-row pixel chunks; chunk 6 is the 16-row leftover
PIX_CH = [(s, min(128, NPIX - s)) for s in range(0, NPIX, 128)]  # 7 chunks
H_CH = [(s, min(128, HID - s)) for s in range(0, HID, 128)]  # 3 chunks


def _legalize_single_wait(nc):
    """This walrus build accepts only one sync-wait per instruction; move
    extra waits onto NoOps inserted just before, on the same engine."""
    n = 0
    for fn in nc.m.functions:
        for bb in fn.blocks:
            new_insts = []
            for inst in bb.instructions:
                si = inst.sync_info
                if si is not None and si.on_wait and len(si.on_wait) > 1:
                    waits = list(si.on_wait)
                    for w in waits[:-1]:
                        nop = mybir.InstNoOp(
                            name=f"{inst.name}-w{n}",
                            sync_info=mybir.SyncInfo(on_wait=[w], on_update=[]),
                            bass_nofuse=True,
                            engine=inst.engine,
                        )
                        n += 1
                        nc.register_instruction(nop, overwrite=True)
                        new_insts.append(nop)
                    inst.sync_info = mybir.SyncInfo(
                        on_wait=[waits[-1]], on_update=list(si.on_update)
                    )
                new_insts.append(inst)
            bb.instructions = new_insts
    return n


def _emit(ctx: ExitStack, tc: tile.TileContext, xt, x6_d, wpk_d, bpk_d, out):
    nc = tc.nc

    const = ctx.enter_context(tc.tile_pool(name="const", bufs=1))
    ps1 = ctx.enter_context(tc.tile_pool(name="ps1", bufs=3, space="PSUM"))
    ps2p = ctx.enter_context(tc.tile_pool(name="ps2p", bufs=1, space="PSUM"))
    ps3p = ctx.enter_context(tc.tile_pool(name="ps3p", bufs=2, space="PSUM"))
    hp_ = ctx.enter_context(tc.tile_pool(name="hp", bufs=2))
    obp = ctx.enter_context(tc.tile_pool(name="obp", bufs=4))

    # PE warmup operand: zeros (values are irrelevant for the HAM clock
    # gate; matmuls just need to keep the array busy ~3.4us).
    wz = const.tile([128, 128], BF16, name="wz")
    nc.vector.memset(wz[:, :], 0)

    # --- replicated weights on the Activation HW-DGE ring, split so the
    # first fc1 matmuls are gated by one 77KB chunk slice, not the whole
    # 775KB pack. wpk layout (host-packed, bf16): cols [0,1800) = w1p
    # chunks 0-5, [1800,2100) = w1p6 (replicated at partition offsets
    # 0/32/64), [2100,3000) = w2 chunks, [3000,3030) = w3 chunks. ---
    wpk = const.tile([128, 3030], BF16, name="wpk")
    nc.scalar.dma_start(wpk[:, 0:900], wpk_d[:, 0:900])
    nc.scalar.dma_start(wpk[:, 900:1800], wpk_d[:, 900:1800])
    # bias pack (f32): cols 0-2 = b1 chunks, 3-5 = b2 chunks, 6 = b3
    bpk = const.tile([128, 7], F32, name="bpk")
    nc.scalar.dma_start(bpk[:, :], bpk_d[:, :])
    nc.scalar.dma_start(wpk[:, 1800:2100], wpk_d[:, 1800:2100])
    nc.scalar.dma_start(wpk[:, 2100:3030], wpk_d[:, 2100:3030])
    w1p = [wpk[0:pw, pc * HID : pc * HID + HID] for pc, (p0, pw) in enumerate(PIX_CH[:NFULL])]
    w1p6 = wpk[0:80, NFULL * HID : NFULL * HID + HID]
    w2s = [wpk[0:hp, 2100 + hc * HID : 2100 + (hc + 1) * HID] for hc, (h0, hp) in enumerate(H_CH)]
    w3s = [wpk[0:hp, 3000 + hc * NCLS : 3000 + (hc + 1) * NCLS] for hc, (h0, hp) in enumerate(H_CH)]
    b1s = [bpk[0:hp, hc : hc + 1] for hc, (h0, hp) in enumerate(H_CH)]
    b2s = [bpk[0:hp, 3 + hc : 4 + hc] for hc, (h0, hp) in enumerate(H_CH)]
    b3s = bpk[0:NCLS, 6:7]

    # --- whole-x SBUF residency, big-packet layout: the host packs each
    # batch tile's six full pixel chunks side by side per partition row
    # (xt row t*128+p holds [x[pc*128+p, t*512:(t+1)*512] for pc in 0..5]
    # = 6KB contiguous), so every DMA packet is 6KB/3KB and the SP ring
    # streams at ~300GB/s instead of ~50GB/s with 1KB packets.  Two DMAs
    # per tile (chunks 0-2, 3-5) so tile-0 compute starts on the first
    # half.  The 16-row leftover ships whole on SWDGE (8KB rows). ---
    xsegs = []
    for t in range(NBT):
        xga = const.tile([128, 3 * BT], BF16, name=f"xa_{t}")
        nc.sync.dma_start(xga[:, :], xt[t * 128 : (t + 1) * 128, 0 : 3 * BT])
        xgb = const.tile([128, 3 * BT], BF16, name=f"xb_{t}")
        nc.sync.dma_start(xgb[:, :], xt[t * 128 : (t + 1) * 128, 3 * BT : 6 * BT])
        xsegs.append(
            [xga[:, pc * BT : (pc + 1) * BT] for pc in range(3)]
            + [xgb[:, pc * BT : (pc + 1) * BT] for pc in range(3)]
        )
        if t == 0:
            x6t = const.tile([16, B_CORE], BF16, name="x6t")
            nc.gpsimd.dma_start(x6t[:, :], x6_d[:, :])

    # warmup burst emitted after the DMA kickoffs so the PE has work while
    # they land: ~4us of junk matmuls release the HAM clock gate so real
    # compute starts at 2.4GHz right as the first x tile arrives.
    warm = ps1.tile([128, 512], F32, name="warm", tag="f1")
    for _ in range(38):
        nc.tensor.matmul(
            warm[0:128, 0:128], wz[:, 0:128], wz[:, 0:128],
            start=True, stop=True,
        )

    # --- main batch loop (x fully resident; no per-tile loads) ---
    def compute(t, c0, off, n):
        """fc1->fc2->fc3->store for batch columns [off, off+n) of one tile."""
        xs = xsegs[t] + [x6t[:, c0 : c0 + BT]]
        # fc1: relu(x @ W1p + b1), output hidden-major [300, n]; each
        # h-group's 7 matmuls stay bank-contiguous and its ACT eviction
        # starts while the next group runs on the PE.
        h1 = []
        for hc, (h0, hp) in enumerate(H_CH):
            ps = ps1.tile([128, 512], F32, name="psa", tag="f1")
            for pc, (p0, pw) in enumerate(PIX_CH):
                nc.tensor.matmul(
                    ps[0:hp, 0:n],
                    w1p[pc][0:pw, h0 : h0 + hp] if pc < NFULL
                    else w1p6[0:16, h0 : h0 + hp],
                    xs[pc][0:pw, off : off + n],
                    start=(pc == 0),
                    stop=(pc == len(PIX_CH) - 1),
                )
            h = hp_.tile([hp, BT], BF16, name=f"h1_{hc}", tag=f"h1_{hc}")
            nc.scalar.activation(
                h[:, 0:n], ps[0:hp, 0:n], AFT.Relu, bias=b1s[hc][:, :]
            )
            h1.append(h)

        # fc2: relu(h1 @ W2 + b2) — m-outer: consecutive matmuls share a
        # PSUM bank, which keeps LDWEIGHTS hidden (bank switches expose it)
        ps2 = [
            ps2p.tile([128, 512], F32, name=f"ps2_{g}", tag=f"g{g}")
            for g in range(len(H_CH))
        ]
        for hc2, (g0, gp) in enumerate(H_CH):
            for hc, (h0, hp) in enumerate(H_CH):
                nc.tensor.matmul(
                    ps2[hc2][0:gp, 0:n],
                    w2s[hc][0:hp, g0 : g0 + gp],
                    h1[hc][0:hp, 0:n],
                    start=(hc == 0),
                    stop=(hc == len(H_CH) - 1),
                )
        # h2 evictions on DVE (bias-add + relu) to keep ScalarE short
        h2 = []
        for hc2, (g0, gp) in enumerate(H_CH):
            h = hp_.tile([gp, BT], BF16, name=f"h2_{hc2}", tag=f"h2_{hc2}")
            nc.vector.tensor_scalar(
                h[:, 0:n], ps2[hc2][0:gp, 0:n], b2s[hc2][:, :], 0.0,
                mybir.AluOpType.add, mybir.AluOpType.max,
            )
            h2.append(h)

        # fc3: h2 @ W3 + b3 -> [10, n] (10-col stationary, LDW ~free);
        # store hidden-major — the host un-transposes.
        ps = ps3p.tile([NCLS, 512], F32, name="ps3", tag="f3")
        for hc, (h0, hp) in enumerate(H_CH):
            nc.tensor.matmul(
                ps[0:NCLS, 0:n],
                w3s[hc][0:hp, 0:NCLS],
                h2[hc][0:hp, 0:n],
                start=(hc == 0),
                stop=(hc == len(H_CH) - 1),
            )
        ob = obp.tile([NCLS, BT], F32, name="ob", tag="ob")
        nc.scalar.activation(
            ob[:, 0:n], ps[0:NCLS, 0:n], AFT.Identity, bias=b3s[:, :]
        )
        nc.sync.dma_start(out[:, c0 + off : c0 + off + n], ob[:, 0:n])

    for t in range(NBT):
        c0 = t * BT
        if t == NBT - 1:
            # split the last tile to shorten the serial tail chain
            compute(t, c0, 0, 256)
            compute(t, c0, 256, 256)
        else:
            compute(t, c0, 0, BT)


def _fold_w1(conv_w: np.ndarray, W1: np.ndarray) -> np.ndarray:
    """W1' = C @ W1 via the 9-tap sparse form: 9 scaled slice-adds."""
    W1m = W1.reshape(OHW, OHW, HID)
    out = np.zeros((IMG, IMG, HID), np.float32)
    for dy in range(3):
        for dx in range(3):
            out[dy : dy + OHW, dx : dx + OHW, :] += conv_w[dy, dx] * W1m
    return out.reshape(NPIX, HID)


_NC_CACHE: list = []


def _get_nc():
    if _NC_CACHE:
        return _NC_CACHE[0]
    nc = bass.Bass("TRN2", target_bir_lowering=False, debug=False)
    # xt rows t*128+p hold tile t's six full pixel chunks side by side:
    # [x[pc*128+p, t*512:(t+1)*512] for pc in 0..5] = 6KB contiguous.
    xt = nc.dram_tensor("xt", [NBT * 128, 6 * BT], BF16, kind="ExternalInput").ap()
    x6 = nc.dram_tensor("x6", [16, B_CORE], BF16, kind="ExternalInput").ap()
    wpk = nc.dram_tensor("wpk", [128, 3030], BF16, kind="ExternalInput").ap()
    bpk = nc.dram_tensor("bpk", [128, 7], F32, kind="ExternalInput").ap()
    out = nc.dram_tensor("out", [NCLS, B_CORE], F32, kind="ExternalOutput").ap()
    with tile.TileContext(nc) as tc:
        with ExitStack() as ctx:
            _emit(ctx, tc, xt, x6, wpk, bpk, out)
    _legalize_single_wait(nc)
    _NC_CACHE.append(nc)
    return nc


def _in_maps(inputs: dict) -> list:
    x = np.asarray(inputs["x"], dtype=np.float32)
    assert x.shape == (B_FULL, NPIX), x.shape
    bf = ml_dtypes.bfloat16
    # pixel-major per-core layout: [8, 784, 4096] bf16 (zero-FLOP reshape)
    xtp = x.reshape(N_CORES, B_CORE, NPIX).transpose(0, 2, 1).astype(bf)
    # big-packet tile layout: [core][t*128+p, pc*512+j] = xtp[core, pc*128+p,
    # t*512+j] -> every DMA packet is a 6KB contiguous row
    xta = np.ascontiguousarray(
        xtp[:, :768, :]
        .reshape(N_CORES, NFULL, 128, NBT, BT)
        .transpose(0, 3, 2, 1, 4)
        .reshape(N_CORES, NBT * 128, NFULL * BT)
    )
    x6t = np.ascontiguousarray(xtp[:, 768:784, :])
    w1f = _fold_w1(
        np.asarray(inputs["conv_w"], np.float32),
        np.asarray(inputs["W1"], np.float32),
    ).astype(bf)
    W2 = np.asarray(inputs["W2"], np.float32)
    W3 = np.asarray(inputs["W3"], np.float32)
    # packed weight tile: w1p chunks 0-5 | w1p6 (replicated) | w2 | w3
    wpk = np.zeros((128, 3030), bf)
    for pc in range(NFULL):
        wpk[:, pc * HID : (pc + 1) * HID] = w1f[pc * 128 : (pc + 1) * 128]
    for r in range(3):
        wpk[32 * r : 32 * r + 16, NFULL * HID : NFULL * HID + HID] = w1f[768:784]
    for hc, (h0, hp) in enumerate(H_CH):
        wpk[0:hp, 2100 + hc * HID : 2100 + (hc + 1) * HID] = W2[h0 : h0 + hp].astype(bf)
        wpk[0:hp, 3000 + hc * NCLS : 3000 + (hc + 1) * NCLS] = W3[h0 : h0 + hp].astype(bf)
    bpk = np.zeros((128, 7), np.float32)
    b1 = np.asarray(inputs["b1"], np.float32)
    b2 = np.asarray(inputs["b2"], np.float32)
    for hc, (h0, hp) in enumerate(H_CH):
        bpk[0:hp, hc] = b1[h0 : h0 + hp]
        bpk[0:hp, 3 + hc] = b2[h0 : h0 + hp]
    bpk[0:NCLS, 6] = np.asarray(inputs["b3"], np.float32)
    common = {"wpk": wpk, "bpk": bpk}
    return [{"xt": xta[c], "x6": x6t[c], **common} for c in range(N_CORES)]


def kernel(**inputs) -> np.ndarray:
    nc = _get_nc()
    res = run_bass_kernel_spmd(nc, _in_maps(inputs), list(range(N_CORES)))
    return np.concatenate(
        [res.results[c]["out"].T for c in range(N_CORES)], axis=0
    )


if __name__ == "__main__":
    rng = np.random.default_rng(0)
    ins = {
        "x": rng.standard_normal((B_FULL, NPIX), dtype=np.float32),
        "conv_w": rng.standard_normal((3, 3), dtype=np.float32) * 0.1,
        "W1": rng.standard_normal((FLAT, HID), dtype=np.float32) * 0.04,
        "b1": np.zeros(HID, np.float32),
        "W2": rng.standard_normal((HID, HID), dtype=np.float32) * 0.06,
        "b2": np.zeros(HID, np.float32),
        "W3": rng.standard_normal((HID, NCLS), dtype=np.float32) * 0.06,
        "b3": np.zeros(NCLS, np.float32),
    }
    y = kernel(**ins)
    # numpy reference with explicit conv
    from numpy.lib.stride_tricks import sliding_window_view

    img = ins["x"].reshape(-1, IMG, IMG)
    win = sliding_window_view(img, (3, 3), axis=(1, 2))
    conv = np.einsum("bijkl,kl->bij", win, ins["conv_w"]).reshape(-1, FLAT)
    h = np.maximum(conv @ ins["W1"] + ins["b1"], 0)
    h = np.maximum(h @ ins["W2"] + ins["b2"], 0)
    ref = h @ ins["W3"] + ins["b3"]
    err = np.abs(y - ref).max() / (np.abs(ref).max() + 1e-9)
    print("max rel err vs numpy:", err)



# revision 13
# speedup vs baseline: 1.1401x; 1.1401x over previous
"""Trainium2 Bass kernel for DigitConvolutionalModel forward pass.

Model: x[B,784] -> 3x3 valid conv (single channel) -> flatten[676]
       -> relu(.@W1+b1) -> relu(.@W2+b2) -> .@W3+b3 -> [B,10]

Strategy (v4):
  - Pure data parallel: batch 32768 sharded 8 ways (4096 rows/core);
    weights replicated.
  - conv folds into fc1 (host-side 9-tap sparse weight fold, ~0.02% of
    model FLOPs): fc1 contracts K=784 of pixel-major x against
    W1' = C @ W1. All batch compute runs on device in bf16 (fp32 PSUM).
  - Host supplies x pixel-major bf16 ([784, 4096] per core) and reads the
    output back pixel-major ([10, 4096] per core) — zero-FLOP layout
    changes that remove every on-device transpose.
  - fc1's K=16 leftover chunk (784 = 6*128 + 16) is packed: the three
    h-group tail matmuls run concurrently in disjoint 32-row PE groups
    (tile_position), with x[768:784] and W1'[768:784] replicated at
    partition offsets 0/32/64.
  - fc3 keeps hidden-major [10, 512] output (stationary = W3 chunks of
    only 10 columns -> LDWEIGHTS ~free); bias fused in the ScalarE
    eviction; the tile DMAs straight out to the [10, 4096] buffer.
  - Input + weight DMAs split across both HW-DGE rings (SP + Activation)
    so the prologue is not serialized behind one ~200 GB/s queue.
"""

import sys

for _p in (
    "/opt/trn_rl_repo",
    "/root/.axon_site",
    "/root/.axon_site/_ro/trn_rl_repo",
    "/root/.axon_site/_ro/pypackages",
):
    if _p not in sys.path:
        sys.path.append(_p)

from contextlib import ExitStack

import numpy as np
import ml_dtypes

import concourse.bass as bass
import concourse.tile as tile
from concourse import mybir
from concourse.bass_utils import run_bass_kernel_spmd

F32 = mybir.dt.float32
BF16 = mybir.dt.bfloat16
AFT = mybir.ActivationFunctionType

B_FULL = 32768
N_CORES = 8
B_CORE = B_FULL // N_CORES  # 4096
IMG = 28
OHW = 26
FLAT = OHW * OHW  # 676
NPIX = IMG * IMG  # 784
HID = 300
NCLS = 10

BT = 512  # batch tile (matmul moving free dim)
NBT = B_CORE // BT  # 8

NFULL = 6  # full 128-row pixel chunks; chunk 6 is the 16-row leftover
PIX_CH = [(s, min(128, NPIX - s)) for s in range(0, NPIX, 128)]  # 7 chunks
H_CH = [(s, min(128, HID - s)) for s in range(0, HID, 128)]  # 3 chunks


def _legalize_single_wait(nc):
    """This walrus build accepts only one sync-wait per instruction; move
    extra waits onto NoOps inserted just before, on the same engine."""
    n = 0
    for fn in nc.m.functions:
        for bb in fn.blocks:
            new_insts = []
            for inst in bb.instructions:
                si = inst.sync_info
                if si is not None and si.on_wait and len(si.on_wait) > 1:
                    waits = list(si.on_wait)
                    for w in waits[:-1]:
                        nop = mybir.InstNoOp(
                            name=f"{inst.name}-w{n}",
                            sync_info=mybir.SyncInfo(on_wait=[w], on_update=[]),
                            bass_nofuse=True,
                            engine=inst.engine,
                        )
                        n += 1
                        nc.register_instruction(nop, overwrite=True)
                        new_insts.append(nop)
                    inst.sync_info = mybir.SyncInfo(
                        on_wait=[waits[-1]], on_update=list(si.on_update)
                    )
                new_insts.append(inst)
            bb.instructions = new_insts
    return n


def _emit(ctx: ExitStack, tc: tile.TileContext, xt, x6_d, wpk_d, bpk_d, out):
    nc = tc.nc

    const = ctx.enter_context(tc.tile_pool(name="const", bufs=1))
    ps1 = ctx.enter_context(tc.tile_pool(name="ps1", bufs=3, space="PSUM"))
    ps2p = ctx.enter_context(tc.tile_pool(name="ps2p", bufs=1, space="PSUM"))
    ps3p = ctx.enter_context(tc.tile_pool(name="ps3p", bufs=2, space="PSUM"))
    hp_ = ctx.enter_context(tc.tile_pool(name="hp", bufs=2))
    obp = ctx.enter_context(tc.tile_pool(name="obp", bufs=4))

    # PE warmup operand: zeros (values are irrelevant for the HAM clock
    # gate; matmuls just need to keep the array busy ~3.4us).
    wz = const.tile([128, 128], BF16, name="wz")
    nc.vector.memset(wz[:, :], 0)

    # --- replicated weights on the Activation HW-DGE ring, split so the
    # first fc1 matmuls are gated by one 77KB chunk slice, not the whole
    # 775KB pack. wpk layout (host-packed, bf16): cols [0,1800) = w1p
    # chunks 0-5, [1800,2100) = w1p6 (replicated at partition offsets
    # 0/32/64), [2100,3000) = w2 chunks, [3000,3030) = w3 chunks. ---
    # wpk layout (host-packed, bf16): cols [0,1800) = w1p chunks 0-5,
    # [1800,2100) = w1p6 (replicated at partition offsets 0/32/64),
    # [2100,3000) = w2 chunks, [3000,3030) = w3 chunks, [3030,3037) =
    # biases (b1 x3, b2 x3, b3) so no separate tiny-packet bias DMA.
    wpk = const.tile([128, 3037], BF16, name="wpk")
    nc.scalar.dma_start(wpk[:, 0:900], wpk_d[:, 0:900])
    nc.scalar.dma_start(wpk[:, 900:1800], wpk_d[:, 900:1800])
    nc.scalar.dma_start(wpk[:, 1800:2100], wpk_d[:, 1800:2100])
    nc.scalar.dma_start(wpk[:, 2100:3037], wpk_d[:, 2100:3037])
    w1p = [wpk[0:pw, pc * HID : pc * HID + HID] for pc, (p0, pw) in enumerate(PIX_CH[:NFULL])]
    w1p6 = wpk[0:80, NFULL * HID : NFULL * HID + HID]
    w2s = [wpk[0:hp, 2100 + hc * HID : 2100 + (hc + 1) * HID] for hc, (h0, hp) in enumerate(H_CH)]
    w3s = [wpk[0:hp, 3000 + hc * NCLS : 3000 + (hc + 1) * NCLS] for hc, (h0, hp) in enumerate(H_CH)]
    b1s = [wpk[0:hp, 3030 + hc : 3031 + hc] for hc, (h0, hp) in enumerate(H_CH)]
    b2s = [wpk[0:hp, 3033 + hc : 3034 + hc] for hc, (h0, hp) in enumerate(H_CH)]
    b3s = wpk[0:NCLS, 3036:3037]

    # --- whole-x SBUF residency, big-packet layout: the host packs each
    # batch tile's six full pixel chunks side by side per partition row
    # (xt row t*128+p holds [x[pc*128+p, t*512:(t+1)*512] for pc in 0..5]
    # = 6KB contiguous), so every DMA packet is 6KB/3KB and the SP ring
    # streams at ~300GB/s instead of ~50GB/s with 1KB packets.  Two DMAs
    # per tile (chunks 0-2, 3-5) so tile-0 compute starts on the first
    # half.  The 16-row leftover ships whole on SWDGE (8KB rows). ---
    xsegs = []
    for t in range(NBT):
        xga = const.tile([128, 3 * BT], BF16, name=f"xa_{t}")
        nc.sync.dma_start(xga[:, :], xt[t * 128 : (t + 1) * 128, 0 : 3 * BT])
        xgb = const.tile([128, 3 * BT], BF16, name=f"xb_{t}")
        nc.sync.dma_start(xgb[:, :], xt[t * 128 : (t + 1) * 128, 3 * BT : 6 * BT])
        xsegs.append(
            [xga[:, pc * BT : (pc + 1) * BT] for pc in range(3)]
            + [xgb[:, pc * BT : (pc + 1) * BT] for pc in range(3)]
        )
        if t == 0:
            x6t = const.tile([16, B_CORE], BF16, name="x6t")
            nc.gpsimd.dma_start(x6t[:, :], x6_d[:, :])

    # warmup burst emitted after the DMA kickoffs so the PE has work while
    # they land: ~4us of junk matmuls release the HAM clock gate so real
    # compute starts at 2.4GHz right as the first x tile arrives.
    warm = ps1.tile([128, 512], F32, name="warm", tag="f1")
    for _ in range(38):
        nc.tensor.matmul(
            warm[0:128, 0:128], wz[:, 0:128], wz[:, 0:128],
            start=True, stop=True,
        )

    # --- main batch loop (x fully resident; no per-tile loads) ---
    def compute(t, c0, off, n):
        """fc1->fc2->fc3->store for batch columns [off, off+n) of one tile."""
        xs = xsegs[t] + [x6t[:, c0 : c0 + BT]]
        # fc1: relu(x @ W1p + b1), output hidden-major [300, n]; each
        # h-group's 7 matmuls stay bank-contiguous and its ACT eviction
        # starts while the next group runs on the PE.
        h1 = []
        for hc, (h0, hp) in enumerate(H_CH):
            ps = ps1.tile([128, 512], F32, name="psa", tag="f1")
            for pc, (p0, pw) in enumerate(PIX_CH):
                nc.tensor.matmul(
                    ps[0:hp, 0:n],
                    w1p[pc][0:pw, h0 : h0 + hp] if pc < NFULL
                    else w1p6[0:16, h0 : h0 + hp],
                    xs[pc][0:pw, off : off + n],
                    start=(pc == 0),
                    stop=(pc == len(PIX_CH) - 1),
                )
            h = hp_.tile([hp, BT], BF16, name=f"h1_{hc}", tag=f"h1_{hc}")
            nc.scalar.activation(
                h[:, 0:n], ps[0:hp, 0:n], AFT.Relu, bias=b1s[hc][:, :]
            )
            h1.append(h)

        # fc2: relu(h1 @ W2 + b2) — m-outer: consecutive matmuls share a
        # PSUM bank, which keeps LDWEIGHTS hidden (bank switches expose it)
        ps2 = [
            ps2p.tile([128, 512], F32, name=f"ps2_{g}", tag=f"g{g}")
            for g in range(len(H_CH))
        ]
        for hc2, (g0, gp) in enumerate(H_CH):
            for hc, (h0, hp) in enumerate(H_CH):
                nc.tensor.matmul(
                    ps2[hc2][0:gp, 0:n],
                    w2s[hc][0:hp, g0 : g0 + gp],
                    h1[hc][0:hp, 0:n],
                    start=(hc == 0),
                    stop=(hc == len(H_CH) - 1),
                )
        # h2 evictions on DVE (bias-add + relu) to keep ScalarE short
        h2 = []
        for hc2, (g0, gp) in enumerate(H_CH):
            h = hp_.tile([gp, BT], BF16, name=f"h2_{hc2}", tag=f"h2_{hc2}")
            nc.vector.tensor_scalar(
                h[:, 0:n], ps2[hc2][0:gp, 0:n], b2s[hc2][:, :], 0.0,
                mybir.AluOpType.add, mybir.AluOpType.max,
            )
            h2.append(h)

        # fc3: h2 @ W3 + b3 -> [10, n] (10-col stationary, LDW ~free);
        # store hidden-major — the host un-transposes.
        ps = ps3p.tile([NCLS, 512], F32, name="ps3", tag="f3")
        for hc, (h0, hp) in enumerate(H_CH):
            nc.tensor.matmul(
                ps[0:NCLS, 0:n],
                w3s[hc][0:hp, 0:NCLS],
                h2[hc][0:hp, 0:n],
                start=(hc == 0),
                stop=(hc == len(H_CH) - 1),
            )
        ob = obp.tile([NCLS, BT], F32, name="ob", tag="ob")
        nc.scalar.activation(
            ob[:, 0:n], ps[0:NCLS, 0:n], AFT.Identity, bias=b3s[:, :]
        )
        nc.sync.dma_start(out[:, c0 + off : c0 + off + n], ob[:, 0:n])

    for t in range(NBT):
        c0 = t * BT
        if t == NBT - 1:
            # split the last tile to shorten the serial tail chain
            compute(t, c0, 0, 256)
            compute(t, c0, 256, 256)
        else:
            compute(t, c0, 0, BT)


def _fold_w1(conv_w: np.ndarray, W1: np.ndarray) -> np.ndarray:
    """W1' = C @ W1 via the 9-tap sparse form: 9 scaled slice-adds."""
    W1m = W1.reshape(OHW, OHW, HID)
    out = np.zeros((IMG, IMG, HID), np.float32)
    for dy in range(3):
        for dx in range(3):
            out[dy : dy + OHW, dx : dx + OHW, :] += conv_w[dy, dx] * W1m
    return out.reshape(NPIX, HID)


_NC_CACHE: list = []


def _get_nc():
    if _NC_CACHE:
        return _NC_CACHE[0]
    nc = bass.Bass("TRN2", target_bir_lowering=False, debug=False)
    # xt rows t*128+p hold tile t's six full pixel chunks side by side:
    # [x[pc*128+p, t*512:(t+1)*512] for pc in 0..5] = 6KB contiguous.
    xt = nc.dram_tensor("xt", [NBT * 128, 6 * BT], BF16, kind="ExternalInput").ap()
    x6 = nc.dram_tensor("x6", [16, B_CORE], BF16, kind="ExternalInput").ap()
    wpk = nc.dram_tensor("wpk", [128, 3030], BF16, kind="ExternalInput").ap()
    bpk = nc.dram_tensor("bpk", [128, 7], F32, kind="ExternalInput").ap()
    out = nc.dram_tensor("out", [NCLS, B_CORE], F32, kind="ExternalOutput").ap()
    with tile.TileContext(nc) as tc:
        with ExitStack() as ctx:
            _emit(ctx, tc, xt, x6, wpk, bpk, out)
    _legalize_single_wait(nc)
    _NC_CACHE.append(nc)
    return nc


def _in_maps(inputs: dict) -> list:
    x = np.asarray(inputs["x"], dtype=np.float32)
    assert x.shape == (B_FULL, NPIX), x.shape
    bf = ml_dtypes.bfloat16
    # pixel-major per-core layout: [8, 784, 4096] bf16 (zero-FLOP reshape)
    xtp = x.reshape(N_CORES, B_CORE, NPIX).transpose(0, 2, 1).astype(bf)
    # big-packet tile layout: [core][t*128+p, pc*512+j] = xtp[core, pc*128+p,
    # t*512+j] -> every DMA packet is a 6KB contiguous row
    xta = np.ascontiguousarray(
        xtp[:, :768, :]
        .reshape(N_CORES, NFULL, 128, NBT, BT)
        .transpose(0, 3, 2, 1, 4)
        .reshape(N_CORES, NBT * 128, NFULL * BT)
    )
    x6t = np.ascontiguousarray(xtp[:, 768:784, :])
    w1f = _fold_w1(
        np.asarray(inputs["conv_w"], np.float32),
        np.asarray(inputs["W1"], np.float32),
    ).astype(bf)
    W2 = np.asarray(inputs["W2"], np.float32)
    W3 = np.asarray(inputs["W3"], np.float32)
    # packed weight tile: w1p chunks 0-5 | w1p6 (replicated) | w2 | w3
    wpk = np.zeros((128, 3030), bf)
    for pc in range(NFULL):
        wpk[:, pc * HID : (pc + 1) * HID] = w1f[pc * 128 : (pc + 1) * 128]
    for r in range(3):
        wpk[32 * r : 32 * r + 16, NFULL * HID : NFULL * HID + HID] = w1f[768:784]
    for hc, (h0, hp) in enumerate(H_CH):
        wpk[0:hp, 2100 + hc * HID : 2100 + (hc + 1) * HID] = W2[h0 : h0 + hp].astype(bf)
        wpk[0:hp, 3000 + hc * NCLS : 3000 + (hc + 1) * NCLS] = W3[h0 : h0 + hp].astype(bf)
    bpk = np.zeros((128, 7), np.float32)
    b1 = np.asarray(inputs["b1"], np.float32)
    b2 = np.asarray(inputs["b2"], np.float32)
    for hc, (h0, hp) in enumerate(H_CH):
        bpk[0:hp, hc] = b1[h0 : h0 + hp]
        bpk[0:hp, 3 + hc] = b2[h0 : h0 + hp]
    bpk[0:NCLS, 6] = np.asarray(inputs["b3"], np.float32)
    common = {"wpk": wpk, "bpk": bpk}
    return [{"xt": xta[c], "x6": x6t[c], **common} for c in range(N_CORES)]


def kernel(**inputs) -> np.ndarray:
    nc = _get_nc()
    res = run_bass_kernel_spmd(nc, _in_maps(inputs), list(range(N_CORES)))
    return np.concatenate(
        [res.results[c]["out"].T for c in range(N_CORES)], axis=0
    )


if __name__ == "__main__":
    rng = np.random.default_rng(0)
    ins = {
        "x": rng.standard_normal((B_FULL, NPIX), dtype=np.float32),
        "conv_w": rng.standard_normal((3, 3), dtype=np.float32) * 0.1,
        "W1": rng.standard_normal((FLAT, HID), dtype=np.float32) * 0.04,
        "b1": np.zeros(HID, np.float32),
        "W2": rng.standard_normal((HID, HID), dtype=np.float32) * 0.06,
        "b2": np.zeros(HID, np.float32),
        "W3": rng.standard_normal((HID, NCLS), dtype=np.float32) * 0.06,
        "b3": np.zeros(NCLS, np.float32),
    }
    y = kernel(**ins)
    # numpy reference with explicit conv
    from numpy.lib.stride_tricks import sliding_window_view

    img = ins["x"].reshape(-1, IMG, IMG)
    win = sliding_window_view(img, (3, 3), axis=(1, 2))
    conv = np.einsum("bijkl,kl->bij", win, ins["conv_w"]).reshape(-1, FLAT)
    h = np.maximum(conv @ ins["W1"] + ins["b1"], 0)
    h = np.maximum(h @ ins["W2"] + ins["b2"], 0)
    ref = h @ ins["W3"] + ins["b3"]
    err = np.abs(y - ref).max() / (np.abs(ref).max() + 1e-9)
    print("max rel err vs numpy:", err)



# revision 14
# speedup vs baseline: 1.1693x; 1.0256x over previous
"""Trainium2 Bass kernel for DigitConvolutionalModel forward pass.

Model: x[B,784] -> 3x3 valid conv (single channel) -> flatten[676]
       -> relu(.@W1+b1) -> relu(.@W2+b2) -> .@W3+b3 -> [B,10]

Strategy (v4):
  - Pure data parallel: batch 32768 sharded 8 ways (4096 rows/core);
    weights replicated.
  - conv folds into fc1 (host-side 9-tap sparse weight fold, ~0.02% of
    model FLOPs): fc1 contracts K=784 of pixel-major x against
    W1' = C @ W1. All batch compute runs on device in bf16 (fp32 PSUM).
  - Host supplies x pixel-major bf16 ([784, 4096] per core) and reads the
    output back pixel-major ([10, 4096] per core) — zero-FLOP layout
    changes that remove every on-device transpose.
  - fc1's K=16 leftover chunk (784 = 6*128 + 16) is packed: the three
    h-group tail matmuls run concurrently in disjoint 32-row PE groups
    (tile_position), with x[768:784] and W1'[768:784] replicated at
    partition offsets 0/32/64.
  - fc3 keeps hidden-major [10, 512] output (stationary = W3 chunks of
    only 10 columns -> LDWEIGHTS ~free); bias fused in the ScalarE
    eviction; the tile DMAs straight out to the [10, 4096] buffer.
  - Input + weight DMAs split across both HW-DGE rings (SP + Activation)
    so the prologue is not serialized behind one ~200 GB/s queue.
"""

import sys

for _p in (
    "/opt/trn_rl_repo",
    "/root/.axon_site",
    "/root/.axon_site/_ro/trn_rl_repo",
    "/root/.axon_site/_ro/pypackages",
):
    if _p not in sys.path:
        sys.path.append(_p)

from contextlib import ExitStack

import numpy as np
import ml_dtypes

import concourse.bass as bass
import concourse.tile as tile
from concourse import mybir
from concourse.bass_utils import run_bass_kernel_spmd

F32 = mybir.dt.float32
BF16 = mybir.dt.bfloat16
AFT = mybir.ActivationFunctionType

B_FULL = 32768
N_CORES = 8
B_CORE = B_FULL // N_CORES  # 4096
IMG = 28
OHW = 26
FLAT = OHW * OHW  # 676
NPIX = IMG * IMG  # 784
HID = 300
NCLS = 10

BT = 512  # batch tile (matmul moving free dim)
NBT = B_CORE // BT  # 8

NFULL = 6  # full 128-row pixel chunks; chunk 6 is the 16-row leftover
PIX_CH = [(s, min(128, NPIX - s)) for s in range(0, NPIX, 128)]  # 7 chunks
H_CH = [(s, min(128, HID - s)) for s in range(0, HID, 128)]  # 3 chunks


def _legalize_single_wait(nc):
    """This walrus build accepts only one sync-wait per instruction; move
    extra waits onto NoOps inserted just before, on the same engine."""
    n = 0
    for fn in nc.m.functions:
        for bb in fn.blocks:
            new_insts = []
            for inst in bb.instructions:
                si = inst.sync_info
                if si is not None and si.on_wait and len(si.on_wait) > 1:
                    waits = list(si.on_wait)
                    for w in waits[:-1]:
                        nop = mybir.InstNoOp(
                            name=f"{inst.name}-w{n}",
                            sync_info=mybir.SyncInfo(on_wait=[w], on_update=[]),
                            bass_nofuse=True,
                            engine=inst.engine,
                        )
                        n += 1
                        nc.register_instruction(nop, overwrite=True)
                        new_insts.append(nop)
                    inst.sync_info = mybir.SyncInfo(
                        on_wait=[waits[-1]], on_update=list(si.on_update)
                    )
                new_insts.append(inst)
            bb.instructions = new_insts
    return n


def _emit(ctx: ExitStack, tc: tile.TileContext, xt, x6_d, wpk_d, out):
    nc = tc.nc

    const = ctx.enter_context(tc.tile_pool(name="const", bufs=1))
    ps1 = ctx.enter_context(tc.tile_pool(name="ps1", bufs=3, space="PSUM"))
    ps2p = ctx.enter_context(tc.tile_pool(name="ps2p", bufs=1, space="PSUM"))
    ps3p = ctx.enter_context(tc.tile_pool(name="ps3p", bufs=2, space="PSUM"))
    xp = ctx.enter_context(tc.tile_pool(name="xp", bufs=3))
    hp_ = ctx.enter_context(tc.tile_pool(name="hp", bufs=2))
    obp = ctx.enter_context(tc.tile_pool(name="obp", bufs=4))

    # PE warmup operand: zeros (values are irrelevant for the HAM clock
    # gate; matmuls just need to keep the array busy ~3.4us).
    wz = const.tile([128, 128], BF16, name="wz")
    nc.vector.memset(wz[:, :], 0)

    # --- replicated weights on the Activation HW-DGE ring, split so the
    # first fc1 matmuls are gated by one column slice, not the whole pack.
    # wpk layout (host-packed, bf16): cols [0,1800) = w1p chunks 0-5,
    # [1800,2100) = w1p6 (replicated at partition offsets 0/32/64),
    # [2100,3000) = w2 chunks, [3000,3030) = w3 chunks, [3030,3037) =
    # biases (b1 x3, b2 x3, b3) so no separate tiny-packet bias DMA.
    wpk = const.tile([128, 3037], BF16, name="wpk")
    nc.scalar.dma_start(wpk[:, 0:900], wpk_d[:, 0:900])
    nc.scalar.dma_start(wpk[:, 900:1800], wpk_d[:, 900:1800])
    nc.scalar.dma_start(wpk[:, 1800:2100], wpk_d[:, 1800:2100])
    nc.scalar.dma_start(wpk[:, 2100:3037], wpk_d[:, 2100:3037])
    w1p = [wpk[0:pw, pc * HID : pc * HID + HID] for pc, (p0, pw) in enumerate(PIX_CH[:NFULL])]
    w1p6 = wpk[0:80, NFULL * HID : NFULL * HID + HID]
    w2s = [wpk[0:hp, 2100 + hc * HID : 2100 + (hc + 1) * HID] for hc, (h0, hp) in enumerate(H_CH)]
    w3s = [wpk[0:hp, 3000 + hc * NCLS : 3000 + (hc + 1) * NCLS] for hc, (h0, hp) in enumerate(H_CH)]
    # biases ride in wpk as bf16 (avoids a 128-tiny-packet f32 DMA); one
    # DVE copy up-converts them to the f32 the engines' bias operands need
    bcv = const.tile([128, 7], F32, name="bcv")
    nc.vector.tensor_copy(bcv[:, :], wpk[:, 3030:3037])
    b1s = [bcv[0:hp, hc : hc + 1] for hc, (h0, hp) in enumerate(H_CH)]
    b2s = [bcv[0:hp, 3 + hc : 4 + hc] for hc, (h0, hp) in enumerate(H_CH)]
    b3s = bcv[0:NCLS, 6:7]

    # --- whole-x SBUF residency, big-packet layout: the host packs each
    # batch tile's six full pixel chunks side by side per partition row
    # (xt row t*128+p holds [x[pc*128+p, t*512:(t+1)*512] for pc in 0..5]
    # = 6KB contiguous), so every DMA packet is 6KB/3KB and the SP ring
    # streams at ~300GB/s instead of ~50GB/s with 1KB packets.  Two DMAs
    # per tile (chunks 0-2, 3-5) so tile-0 compute starts on the first
    # half.  The 16-row leftover ships whole on SWDGE (8KB rows). ---
    # x is demand-paced (NOT front-loaded): a sustained full-rate DMA
    # burst trips the chip into P0 and the PE drops from 2.4 to 2.0GHz
    # for the rest of the run (measured: 454ns vs 379ns per N=512 MM).
    # Per-tile loads from a 3-deep pool, 2 tiles ahead, rings alternating
    # per tile, keep the average stream near ~125GB/s.
    xsegs = {}

    def load_tile(t):
        eng = nc.sync if t % 2 == 0 else nc.scalar
        xga = xp.tile([128, 3 * BT], BF16, name=f"xa_{t}", tag="xa")
        eng.dma_start(xga[:, :], xt[t * 128 : (t + 1) * 128, 0 : 3 * BT])
        xgb = xp.tile([128, 3 * BT], BF16, name=f"xb_{t}", tag="xb")
        eng.dma_start(xgb[:, :], xt[t * 128 : (t + 1) * 128, 3 * BT : 6 * BT])
        xsegs[t] = [xga[:, pc * BT : (pc + 1) * BT] for pc in range(3)] + [
            xgb[:, pc * BT : (pc + 1) * BT] for pc in range(3)
        ]

    load_tile(0)
    x6t = const.tile([16, B_CORE], BF16, name="x6t")
    nc.gpsimd.dma_start(x6t[:, :], x6_d[:, :])
    load_tile(1)

    # warmup burst emitted after the DMA kickoffs so the PE has work while
    # they land: ~4us of junk matmuls release the HAM clock gate so real
    # compute starts at 2.4GHz right as the first x tile arrives.
    warm = ps1.tile([128, 512], F32, name="warm", tag="f1")
    for _ in range(38):
        nc.tensor.matmul(
            warm[0:128, 0:128], wz[:, 0:128], wz[:, 0:128],
            start=True, stop=True,
        )

    # --- main batch loop (x fully resident; no per-tile loads) ---
    def fc12(t, c0, off, n):
        """fc1+fc2 for batch columns [off, off+n) of one tile; returns h2."""
        xs = xsegs[t] + [x6t[:, c0 : c0 + BT]]
        # fc1: relu(x @ W1p + b1), output hidden-major [300, n]; each
        # h-group's 7 matmuls stay bank-contiguous and its ACT eviction
        # starts while the next group runs on the PE.
        h1 = []
        for hc, (h0, hp) in enumerate(H_CH):
            ps = ps1.tile([128, 512], F32, name="psa", tag="f1")
            for pc, (p0, pw) in enumerate(PIX_CH):
                nc.tensor.matmul(
                    ps[0:hp, 0:n],
                    w1p[pc][0:pw, h0 : h0 + hp] if pc < NFULL
                    else w1p6[0:16, h0 : h0 + hp],
                    xs[pc][0:pw, off : off + n],
                    start=(pc == 0),
                    stop=(pc == len(PIX_CH) - 1),
                )
            h = hp_.tile([hp, BT], BF16, name=f"h1_{hc}", tag=f"h1_{hc}")
            nc.scalar.activation(
                h[:, 0:n], ps[0:hp, 0:n], AFT.Relu, bias=b1s[hc][:, :]
            )
            h1.append(h)

        # fc2: relu(h1 @ W2 + b2) — m-outer: consecutive matmuls share a
        # PSUM bank, which keeps LDWEIGHTS hidden (bank switches expose it)
        ps2 = [
            ps2p.tile([128, 512], F32, name=f"ps2_{g}", tag=f"g{g}")
            for g in range(len(H_CH))
        ]
        for hc2, (g0, gp) in enumerate(H_CH):
            for hc, (h0, hp) in enumerate(H_CH):
                nc.tensor.matmul(
                    ps2[hc2][0:gp, 0:n],
                    w2s[hc][0:hp, g0 : g0 + gp],
                    h1[hc][0:hp, 0:n],
                    start=(hc == 0),
                    stop=(hc == len(H_CH) - 1),
                )
        # h2 evictions on DVE (bias-add + relu) to keep ScalarE short
        h2 = []
        for hc2, (g0, gp) in enumerate(H_CH):
            h = hp_.tile([gp, BT], BF16, name=f"h2_{hc2}", tag=f"h2_{hc2}")
            nc.vector.tensor_scalar(
                h[:, 0:n], ps2[hc2][0:gp, 0:n], b2s[hc2][:, :], 0.0,
                mybir.AluOpType.add, mybir.AluOpType.max,
            )
            h2.append(h)
        return h2

    def fc3(h2, c0, off, n):
        """fc3: h2 @ W3 + b3 -> [10, n] (10-col stationary, LDW ~free);
        store hidden-major — the host un-transposes.  Software-pipelined:
        runs one tile late so the PE never sits waiting on h2 evictions."""
        ps = ps3p.tile([NCLS, 512], F32, name="ps3", tag="f3")
        for hc, (h0, hp) in enumerate(H_CH):
            nc.tensor.matmul(
                ps[0:NCLS, 0:n],
                w3s[hc][0:hp, 0:NCLS],
                h2[hc][0:hp, 0:n],
                start=(hc == 0),
                stop=(hc == len(H_CH) - 1),
            )
        ob = obp.tile([NCLS, BT], F32, name="ob", tag="ob")
        nc.scalar.activation(
            ob[:, 0:n], ps[0:NCLS, 0:n], AFT.Identity, bias=b3s[:, :]
        )
        nc.sync.dma_start(out[:, c0 + off : c0 + off + n], ob[:, 0:n])

    pend = None
    for t in range(NBT):
        c0 = t * BT
        if t + 2 < NBT:
            load_tile(t + 2)
        halves = ((0, 256), (256, 256)) if t == NBT - 1 else ((0, BT),)
        for off, n in halves:
            h2 = fc12(t, c0, off, n)
            if pend is not None:
                fc3(*pend)
            pend = (h2, c0, off, n)
        xsegs.pop(t)
    fc3(*pend)


def _fold_w1(conv_w: np.ndarray, W1: np.ndarray) -> np.ndarray:
    """W1' = C @ W1 via the 9-tap sparse form: 9 scaled slice-adds."""
    W1m = W1.reshape(OHW, OHW, HID)
    out = np.zeros((IMG, IMG, HID), np.float32)
    for dy in range(3):
        for dx in range(3):
            out[dy : dy + OHW, dx : dx + OHW, :] += conv_w[dy, dx] * W1m
    return out.reshape(NPIX, HID)


_NC_CACHE: list = []


def _get_nc():
    if _NC_CACHE:
        return _NC_CACHE[0]
    nc = bass.Bass("TRN2", target_bir_lowering=False, debug=False)
    # xt rows t*128+p hold tile t's six full pixel chunks side by side:
    # [x[pc*128+p, t*512:(t+1)*512] for pc in 0..5] = 6KB contiguous.
    xt = nc.dram_tensor("xt", [NBT * 128, 6 * BT], BF16, kind="ExternalInput").ap()
    x6 = nc.dram_tensor("x6", [16, B_CORE], BF16, kind="ExternalInput").ap()
    wpk = nc.dram_tensor("wpk", [128, 3037], BF16, kind="ExternalInput").ap()
    out = nc.dram_tensor("out", [NCLS, B_CORE], F32, kind="ExternalOutput").ap()
    with tile.TileContext(nc) as tc:
        with ExitStack() as ctx:
            _emit(ctx, tc, xt, x6, wpk, out)
    _legalize_single_wait(nc)
    _NC_CACHE.append(nc)
    return nc


def _in_maps(inputs: dict) -> list:
    x = np.asarray(inputs["x"], dtype=np.float32)
    assert x.shape == (B_FULL, NPIX), x.shape
    bf = ml_dtypes.bfloat16
    # pixel-major per-core layout: [8, 784, 4096] bf16 (zero-FLOP reshape)
    xtp = x.reshape(N_CORES, B_CORE, NPIX).transpose(0, 2, 1).astype(bf)
    # big-packet tile layout: [core][t*128+p, pc*512+j] = xtp[core, pc*128+p,
    # t*512+j] -> every DMA packet is a 6KB contiguous row
    xta = np.ascontiguousarray(
        xtp[:, :768, :]
        .reshape(N_CORES, NFULL, 128, NBT, BT)
        .transpose(0, 3, 2, 1, 4)
        .reshape(N_CORES, NBT * 128, NFULL * BT)
    )
    x6t = np.ascontiguousarray(xtp[:, 768:784, :])
    w1f = _fold_w1(
        np.asarray(inputs["conv_w"], np.float32),
        np.asarray(inputs["W1"], np.float32),
    ).astype(bf)
    W2 = np.asarray(inputs["W2"], np.float32)
    W3 = np.asarray(inputs["W3"], np.float32)
    # packed weight tile: w1p chunks 0-5 | w1p6 (replicated) | w2 | w3 | biases
    wpk = np.zeros((128, 3037), bf)
    for pc in range(NFULL):
        wpk[:, pc * HID : (pc + 1) * HID] = w1f[pc * 128 : (pc + 1) * 128]
    for r in range(3):
        wpk[32 * r : 32 * r + 16, NFULL * HID : NFULL * HID + HID] = w1f[768:784]
    for hc, (h0, hp) in enumerate(H_CH):
        wpk[0:hp, 2100 + hc * HID : 2100 + (hc + 1) * HID] = W2[h0 : h0 + hp].astype(bf)
        wpk[0:hp, 3000 + hc * NCLS : 3000 + (hc + 1) * NCLS] = W3[h0 : h0 + hp].astype(bf)
    b1 = np.asarray(inputs["b1"], np.float32)
    b2 = np.asarray(inputs["b2"], np.float32)
    for hc, (h0, hp) in enumerate(H_CH):
        wpk[0:hp, 3030 + hc] = b1[h0 : h0 + hp].astype(bf)
        wpk[0:hp, 3033 + hc] = b2[h0 : h0 + hp].astype(bf)
    wpk[0:NCLS, 3036] = np.asarray(inputs["b3"], np.float32).astype(bf)
    common = {"wpk": wpk}
    return [{"xt": xta[c], "x6": x6t[c], **common} for c in range(N_CORES)]


def kernel(**inputs) -> np.ndarray:
    nc = _get_nc()
    res = run_bass_kernel_spmd(nc, _in_maps(inputs), list(range(N_CORES)))
    return np.concatenate(
        [res.results[c]["out"].T for c in range(N_CORES)], axis=0
    )


if __name__ == "__main__":
    rng = np.random.default_rng(0)
    ins = {
        "x": rng.standard_normal((B_FULL, NPIX), dtype=np.float32),
        "conv_w": rng.standard_normal((3, 3), dtype=np.float32) * 0.1,
        "W1": rng.standard_normal((FLAT, HID), dtype=np.float32) * 0.04,
        "b1": np.zeros(HID, np.float32),
        "W2": rng.standard_normal((HID, HID), dtype=np.float32) * 0.06,
        "b2": np.zeros(HID, np.float32),
        "W3": rng.standard_normal((HID, NCLS), dtype=np.float32) * 0.06,
        "b3": np.zeros(NCLS, np.float32),
    }
    y = kernel(**ins)
    # numpy reference with explicit conv
    from numpy.lib.stride_tricks import sliding_window_view

    img = ins["x"].reshape(-1, IMG, IMG)
    win = sliding_window_view(img, (3, 3), axis=(1, 2))
    conv = np.einsum("bijkl,kl->bij", win, ins["conv_w"]).reshape(-1, FLAT)
    h = np.maximum(conv @ ins["W1"] + ins["b1"], 0)
    h = np.maximum(h @ ins["W2"] + ins["b2"], 0)
    ref = h @ ins["W3"] + ins["b3"]
    err = np.abs(y - ref).max() / (np.abs(ref).max() + 1e-9)
    print("max rel err vs numpy:", err)



# revision 18
# speedup vs baseline: 1.1900x; 1.0177x over previous
"""Trainium2 Bass kernel for DigitConvolutionalModel forward pass.

Model: x[B,784] -> 3x3 valid conv (single channel) -> flatten[676]
       -> relu(.@W1+b1) -> relu(.@W2+b2) -> .@W3+b3 -> [B,10]

Strategy (v4):
  - Pure data parallel: batch 32768 sharded 8 ways (4096 rows/core);
    weights replicated.
  - conv folds into fc1 (host-side 9-tap sparse weight fold, ~0.02% of
    model FLOPs): fc1 contracts K=784 of pixel-major x against
    W1' = C @ W1. All batch compute runs on device in bf16 (fp32 PSUM).
  - Host supplies x pixel-major bf16 ([784, 4096] per core) and reads the
    output back pixel-major ([10, 4096] per core) — zero-FLOP layout
    changes that remove every on-device transpose.
  - fc1's K=16 leftover chunk (784 = 6*128 + 16) is packed: the three
    h-group tail matmuls run concurrently in disjoint 32-row PE groups
    (tile_position), with x[768:784] and W1'[768:784] replicated at
    partition offsets 0/32/64.
  - fc3 keeps hidden-major [10, 512] output (stationary = W3 chunks of
    only 10 columns -> LDWEIGHTS ~free); bias fused in the ScalarE
    eviction; the tile DMAs straight out to the [10, 4096] buffer.
  - Input + weight DMAs split across both HW-DGE rings (SP + Activation)
    so the prologue is not serialized behind one ~200 GB/s queue.
"""

import sys

for _p in (
    "/opt/trn_rl_repo",
    "/root/.axon_site",
    "/root/.axon_site/_ro/trn_rl_repo",
    "/root/.axon_site/_ro/pypackages",
):
    if _p not in sys.path:
        sys.path.append(_p)

from contextlib import ExitStack

import numpy as np
import ml_dtypes

import concourse.bass as bass
import concourse.tile as tile
from concourse import mybir
from concourse.bass_utils import run_bass_kernel_spmd

F32 = mybir.dt.float32
BF16 = mybir.dt.bfloat16
AFT = mybir.ActivationFunctionType

B_FULL = 32768
N_CORES = 8
B_CORE = B_FULL // N_CORES  # 4096
IMG = 28
OHW = 26
FLAT = OHW * OHW  # 676
NPIX = IMG * IMG  # 784
HID = 300
NCLS = 10

BT = 512  # batch tile (matmul moving free dim)
NBT = B_CORE // BT  # 8

NFULL = 6  # full 128-row pixel chunks; chunk 6 is the 16-row leftover
PIX_CH = [(s, min(128, NPIX - s)) for s in range(0, NPIX, 128)]  # 7 chunks
H_CH = [(s, min(128, HID - s)) for s in range(0, HID, 128)]  # 3 chunks


def _legalize_single_wait(nc):
    """This walrus build accepts only one sync-wait per instruction; move
    extra waits onto NoOps inserted just before, on the same engine."""
    n = 0
    for fn in nc.m.functions:
        for bb in fn.blocks:
            new_insts = []
            for inst in bb.instructions:
                si = inst.sync_info
                if si is not None and si.on_wait and len(si.on_wait) > 1:
                    waits = list(si.on_wait)
                    for w in waits[:-1]:
                        nop = mybir.InstNoOp(
                            name=f"{inst.name}-w{n}",
                            sync_info=mybir.SyncInfo(on_wait=[w], on_update=[]),
                            bass_nofuse=True,
                            engine=inst.engine,
                        )
                        n += 1
                        nc.register_instruction(nop, overwrite=True)
                        new_insts.append(nop)
                    inst.sync_info = mybir.SyncInfo(
                        on_wait=[waits[-1]], on_update=list(si.on_update)
                    )
                new_insts.append(inst)
            bb.instructions = new_insts
    return n


def _emit(ctx: ExitStack, tc: tile.TileContext, xt, x6_d, wpk_d, out):
    nc = tc.nc

    const = ctx.enter_context(tc.tile_pool(name="const", bufs=1))
    ps1 = ctx.enter_context(tc.tile_pool(name="ps1", bufs=3, space="PSUM"))
    ps2p = ctx.enter_context(tc.tile_pool(name="ps2p", bufs=1, space="PSUM"))
    ps3p = ctx.enter_context(tc.tile_pool(name="ps3p", bufs=2, space="PSUM"))
    xp = ctx.enter_context(tc.tile_pool(name="xp", bufs=4))
    hp_ = ctx.enter_context(tc.tile_pool(name="hp", bufs=2))
    obp = ctx.enter_context(tc.tile_pool(name="obp", bufs=4))

    # PE warmup operand: zeros (values are irrelevant for the HAM clock
    # gate; matmuls just need to keep the array busy ~3.4us).
    wz = const.tile([128, 128], BF16, name="wz")
    nc.vector.memset(wz[:, :], 0)

    # --- replicated weights on the Activation HW-DGE ring, split so the
    # first fc1 matmuls are gated by one column slice, not the whole pack.
    # wpk layout (host-packed, bf16): cols [0,1800) = w1p chunks 0-5,
    # [1800,2100) = w1p6 (replicated at partition offsets 0/32/64),
    # [2100,3000) = w2 chunks, [3000,3030) = w3 chunks, [3030,3037) =
    # biases (b1 x3, b2 x3, b3) so no separate tiny-packet bias DMA.
    wpk = const.tile([128, 3037], BF16, name="wpk")
    nc.scalar.dma_start(wpk[:, 0:900], wpk_d[:, 0:900])
    nc.scalar.dma_start(wpk[:, 900:1800], wpk_d[:, 900:1800])
    nc.scalar.dma_start(wpk[:, 1800:2100], wpk_d[:, 1800:2100])
    nc.scalar.dma_start(wpk[:, 2100:3037], wpk_d[:, 2100:3037])
    w1p = [wpk[0:pw, pc * HID : pc * HID + HID] for pc, (p0, pw) in enumerate(PIX_CH[:NFULL])]
    w1p6 = wpk[0:80, NFULL * HID : NFULL * HID + HID]
    w2s = [wpk[0:hp, 2100 + hc * HID : 2100 + (hc + 1) * HID] for hc, (h0, hp) in enumerate(H_CH)]
    w3s = [wpk[0:hp, 3000 + hc * NCLS : 3000 + (hc + 1) * NCLS] for hc, (h0, hp) in enumerate(H_CH)]
    # biases ride in wpk as bf16 (avoids a 128-tiny-packet f32 DMA); one
    # DVE copy up-converts them to the f32 the engines' bias operands need
    bcv = const.tile([128, 7], F32, name="bcv")
    nc.vector.tensor_copy(bcv[:, :], wpk[:, 3030:3037])
    b1s = [bcv[0:hp, hc : hc + 1] for hc, (h0, hp) in enumerate(H_CH)]
    b2s = [bcv[0:hp, 3 + hc : 4 + hc] for hc, (h0, hp) in enumerate(H_CH)]
    b3s = bcv[0:NCLS, 6:7]

    # --- whole-x SBUF residency, big-packet layout: the host packs each
    # batch tile's six full pixel chunks side by side per partition row
    # (xt row t*128+p holds [x[pc*128+p, t*512:(t+1)*512] for pc in 0..5]
    # = 6KB contiguous), so every DMA packet is 6KB/3KB and the SP ring
    # streams at ~300GB/s instead of ~50GB/s with 1KB packets.  Two DMAs
    # per tile (chunks 0-2, 3-5) so tile-0 compute starts on the first
    # half.  The 16-row leftover ships whole on SWDGE (8KB rows). ---
    # x is demand-paced (NOT front-loaded): a sustained full-rate DMA
    # burst trips the chip into P0 and the PE drops from 2.4 to 2.0GHz
    # for the rest of the run (measured: 454ns vs 379ns per N=512 MM).
    # Per-tile loads from a 3-deep pool, 2 tiles ahead, rings alternating
    # per tile, keep the average stream near ~125GB/s.
    xsegs = {}

    def load_tile(t):
        # halves on different rings so they land concurrently (~1.3us)
        xga = xp.tile([128, 3 * BT], BF16, name=f"xa_{t}", tag="xa")
        nc.sync.dma_start(xga[:, :], xt[t * 128 : (t + 1) * 128, 0 : 3 * BT])
        xgb = xp.tile([128, 3 * BT], BF16, name=f"xb_{t}", tag="xb")
        nc.scalar.dma_start(xgb[:, :], xt[t * 128 : (t + 1) * 128, 3 * BT : 6 * BT])
        xsegs[t] = [xga[:, pc * BT : (pc + 1) * BT] for pc in range(3)] + [
            xgb[:, pc * BT : (pc + 1) * BT] for pc in range(3)
        ]

    load_tile(0)
    x6t = const.tile([16, B_CORE], BF16, name="x6t")
    nc.gpsimd.dma_start(x6t[:, :], x6_d[:, :])
    load_tile(1)

    # warmup burst emitted after the DMA kickoffs so the PE has work while
    # they land: ~4us of junk matmuls release the HAM clock gate so real
    # compute starts at 2.4GHz right as the first x tile arrives.
    warm = ps1.tile([128, 512], F32, name="warm", tag="f1")
    for _ in range(44):
        nc.tensor.matmul(
            warm[0:128, 0:128], wz[:, 0:128], wz[:, 0:128],
            start=True, stop=True,
        )

    # --- main batch loop (x fully resident; no per-tile loads) ---
    def fc12(t, c0, off, n):
        """fc1+fc2 for batch columns [off, off+n) of one tile; returns h2."""
        xs = xsegs[t] + [x6t[:, c0 : c0 + BT]]
        # fc1: relu(x @ W1p + b1), output hidden-major [300, n]; each
        # h-group's 7 matmuls stay bank-contiguous and its ACT eviction
        # starts while the next group runs on the PE.
        h1 = []
        for hc, (h0, hp) in enumerate(H_CH):
            ps = ps1.tile([128, 512], F32, name="psa", tag="f1")
            for pc, (p0, pw) in enumerate(PIX_CH):
                nc.tensor.matmul(
                    ps[0:hp, 0:n],
                    w1p[pc][0:pw, h0 : h0 + hp] if pc < NFULL
                    else w1p6[0:16, h0 : h0 + hp],
                    xs[pc][0:pw, off : off + n],
                    start=(pc == 0),
                    stop=(pc == len(PIX_CH) - 1),
                )
            h = hp_.tile([hp, BT], BF16, name=f"h1_{hc}", tag=f"h1_{hc}")
            nc.scalar.activation(
                h[:, 0:n], ps[0:hp, 0:n], AFT.Relu, bias=b1s[hc][:, :]
            )
            h1.append(h)

        # fc2: relu(h1 @ W2 + b2) — m-outer: consecutive matmuls share a
        # PSUM bank, which keeps LDWEIGHTS hidden (bank switches expose it)
        ps2 = [
            ps2p.tile([128, 512], F32, name=f"ps2_{g}", tag=f"g{g}")
            for g in range(len(H_CH))
        ]
        for hc2, (g0, gp) in enumerate(H_CH):
            for hc, (h0, hp) in enumerate(H_CH):
                nc.tensor.matmul(
                    ps2[hc2][0:gp, 0:n],
                    w2s[hc][0:hp, g0 : g0 + gp],
                    h1[hc][0:hp, 0:n],
                    start=(hc == 0),
                    stop=(hc == len(H_CH) - 1),
                )
        # h2 evictions on DVE (bias-add + relu) to keep ScalarE short
        h2 = []
        for hc2, (g0, gp) in enumerate(H_CH):
            h = hp_.tile([gp, BT], BF16, name=f"h2_{hc2}", tag=f"h2_{hc2}")
            nc.vector.tensor_scalar(
                h[:, 0:n], ps2[hc2][0:gp, 0:n], b2s[hc2][:, :], 0.0,
                mybir.AluOpType.add, mybir.AluOpType.max,
            )
            h2.append(h)
        return h2

    def fc3(h2, c0, off, n):
        """fc3: h2 @ W3 + b3 -> [10, n] (10-col stationary, LDW ~free);
        store hidden-major — the host un-transposes.  Software-pipelined:
        runs one tile late so the PE never sits waiting on h2 evictions."""
        ps = ps3p.tile([NCLS, 512], F32, name="ps3", tag="f3")
        for hc, (h0, hp) in enumerate(H_CH):
            nc.tensor.matmul(
                ps[0:NCLS, 0:n],
                w3s[hc][0:hp, 0:NCLS],
                h2[hc][0:hp, 0:n],
                start=(hc == 0),
                stop=(hc == len(H_CH) - 1),
            )
        ob = obp.tile([NCLS, BT], F32, name="ob", tag="ob")
        nc.scalar.activation(
            ob[:, 0:n], ps[0:NCLS, 0:n], AFT.Identity, bias=b3s[:, :]
        )
        nc.sync.dma_start(out[:, c0 + off : c0 + off + n], ob[:, 0:n])

    load_tile(2)
    pend = None
    for t in range(NBT):
        c0 = t * BT
        if t + 3 < NBT:
            load_tile(t + 3)
        halves = ((0, 256), (256, 256)) if t == NBT - 1 else ((0, BT),)
        for off, n in halves:
            h2 = fc12(t, c0, off, n)
            if pend is not None:
                fc3(*pend)
            pend = (h2, c0, off, n)
        xsegs.pop(t)
    fc3(*pend)


def _fold_w1(conv_w: np.ndarray, W1: np.ndarray) -> np.ndarray:
    """W1' = C @ W1 via the 9-tap sparse form: 9 scaled slice-adds."""
    W1m = W1.reshape(OHW, OHW, HID)
    out = np.zeros((IMG, IMG, HID), np.float32)
    for dy in range(3):
        for dx in range(3):
            out[dy : dy + OHW, dx : dx + OHW, :] += conv_w[dy, dx] * W1m
    return out.reshape(NPIX, HID)


_NC_CACHE: list = []


def _get_nc():
    if _NC_CACHE:
        return _NC_CACHE[0]
    nc = bass.Bass("TRN2", target_bir_lowering=False, debug=False)
    # xt rows t*128+p hold tile t's six full pixel chunks side by side:
    # [x[pc*128+p, t*512:(t+1)*512] for pc in 0..5] = 6KB contiguous.
    xt = nc.dram_tensor("xt", [NBT * 128, 6 * BT], BF16, kind="ExternalInput").ap()
    x6 = nc.dram_tensor("x6", [16, B_CORE], BF16, kind="ExternalInput").ap()
    wpk = nc.dram_tensor("wpk", [128, 3037], BF16, kind="ExternalInput").ap()
    out = nc.dram_tensor("out", [NCLS, B_CORE], F32, kind="ExternalOutput").ap()
    with tile.TileContext(nc) as tc:
        with ExitStack() as ctx:
            _emit(ctx, tc, xt, x6, wpk, out)
    _legalize_single_wait(nc)
    _NC_CACHE.append(nc)
    return nc


def _in_maps(inputs: dict) -> list:
    x = np.asarray(inputs["x"], dtype=np.float32)
    assert x.shape == (B_FULL, NPIX), x.shape
    bf = ml_dtypes.bfloat16
    # pixel-major per-core layout: [8, 784, 4096] bf16 (zero-FLOP reshape)
    xtp = x.reshape(N_CORES, B_CORE, NPIX).transpose(0, 2, 1).astype(bf)
    # big-packet tile layout: [core][t*128+p, pc*512+j] = xtp[core, pc*128+p,
    # t*512+j] -> every DMA packet is a 6KB contiguous row
    xta = np.ascontiguousarray(
        xtp[:, :768, :]
        .reshape(N_CORES, NFULL, 128, NBT, BT)
        .transpose(0, 3, 2, 1, 4)
        .reshape(N_CORES, NBT * 128, NFULL * BT)
    )
    x6t = np.ascontiguousarray(xtp[:, 768:784, :])
    w1f = _fold_w1(
        np.asarray(inputs["conv_w"], np.float32),
        np.asarray(inputs["W1"], np.float32),
    ).astype(bf)
    W2 = np.asarray(inputs["W2"], np.float32)
    W3 = np.asarray(inputs["W3"], np.float32)
    # packed weight tile: w1p chunks 0-5 | w1p6 (replicated) | w2 | w3 | biases
    wpk = np.zeros((128, 3037), bf)
    for pc in range(NFULL):
        wpk[:, pc * HID : (pc + 1) * HID] = w1f[pc * 128 : (pc + 1) * 128]
    for r in range(3):
        wpk[32 * r : 32 * r + 16, NFULL * HID : NFULL * HID + HID] = w1f[768:784]
    for hc, (h0, hp) in enumerate(H_CH):
        wpk[0:hp, 2100 + hc * HID : 2100 + (hc + 1) * HID] = W2[h0 : h0 + hp].astype(bf)
        wpk[0:hp, 3000 + hc * NCLS : 3000 + (hc + 1) * NCLS] = W3[h0 : h0 + hp].astype(bf)
    b1 = np.asarray(inputs["b1"], np.float32)
    b2 = np.asarray(inputs["b2"], np.float32)
    for hc, (h0, hp) in enumerate(H_CH):
        wpk[0:hp, 3030 + hc] = b1[h0 : h0 + hp].astype(bf)
        wpk[0:hp, 3033 + hc] = b2[h0 : h0 + hp].astype(bf)
    wpk[0:NCLS, 3036] = np.asarray(inputs["b3"], np.float32).astype(bf)
    common = {"wpk": wpk}
    return [{"xt": xta[c], "x6": x6t[c], **common} for c in range(N_CORES)]


def kernel(**inputs) -> np.ndarray:
    nc = _get_nc()
    res = run_bass_kernel_spmd(nc, _in_maps(inputs), list(range(N_CORES)))
    return np.concatenate(
        [res.results[c]["out"].T for c in range(N_CORES)], axis=0
    )


if __name__ == "__main__":
    rng = np.random.default_rng(0)
    ins = {
        "x": rng.standard_normal((B_FULL, NPIX), dtype=np.float32),
        "conv_w": rng.standard_normal((3, 3), dtype=np.float32) * 0.1,
        "W1": rng.standard_normal((FLAT, HID), dtype=np.float32) * 0.04,
        "b1": np.zeros(HID, np.float32),
        "W2": rng.standard_normal((HID, HID), dtype=np.float32) * 0.06,
        "b2": np.zeros(HID, np.float32),
        "W3": rng.standard_normal((HID, NCLS), dtype=np.float32) * 0.06,
        "b3": np.zeros(NCLS, np.float32),
    }
    y = kernel(**ins)
    # numpy reference with explicit conv
    from numpy.lib.stride_tricks import sliding_window_view

    img = ins["x"].reshape(-1, IMG, IMG)
    win = sliding_window_view(img, (3, 3), axis=(1, 2))
    conv = np.einsum("bijkl,kl->bij", win, ins["conv_w"]).reshape(-1, FLAT)
    h = np.maximum(conv @ ins["W1"] + ins["b1"], 0)
    h = np.maximum(h @ ins["W2"] + ins["b2"], 0)
    ref = h @ ins["W3"] + ins["b3"]
    err = np.abs(y - ref).max() / (np.abs(ref).max() + 1e-9)
    print("max rel err vs numpy:", err)

